# revision 2
# baseline (speedup 1.0000x reference)
"""Trainium2 Bass kernel for nn_AttentionBranch (sparse GQA attention + RoPE).

Problem (hardcoded): B=1, S=2176, 32 q heads, 8 kv heads, head_dim=128,
mask = causal & (sliding-window-256 | kv < 128 meta prefix), fp32 io.

Sharding: 8 cores; core c owns q heads [4c, 4c+4) and kv head c (GQA group).

Per-core dataflow (SPMD, one Bass program):
  - Inputs ship single-copy (no host half-swap): q [4, D, S], k [D, S].
    The RoPE half-swapped copies are produced on-device by SBUF->SBUF DMA
    partition-swap (spare DMA bandwidth, zero engine cycles).
  - RoPE via 3 DVE ops per tensor (cos / sign-folded-sin tables from host).
  - Block-sparse attention over 128-row q blocks: kv blocks {0, i-2, i-1, i}.
    Scores transposed (kv on partitions): QK matmuls + triangular additive
    masks (via matmul accumulate) write into hole-free 1024-col PSUM group
    tiles; ONE exp per group (~9 ACT calls/head instead of 29) amortizes
    the ~290ns ACTIVATE fixed cost. Row sums via ones-matmul broadcast into
    PSUM, PV via V-stationary matmuls, normalize with
    reciprocal_approx_fast + tensor_mul, output written bf16 (host widens).
"""

import math
import os
from functools import lru_cache

import numpy as np
import ml_dtypes

S = 2176
D = 128
NB = S // 128  # 17 q/kv blocks
HQ_PER_CORE = 4
N_CORES = 8
WINDOW = 256
META = 128
ROPE_BASE = 10000.0
SCALE = 1.0 / math.sqrt(D)

BF16 = ml_dtypes.bfloat16
LAST_RESULT = None

# q-block ranges of the per-head processing pieces (zb/ot accumulators are
# one PSUM bank each, double-buffered => 4 banks; group tiles use the rest).
PIECES = [(0, 3), (4, 7), (8, 11), (12, 15), (16, 16)]


def _strips_for_piece(b0, b1):
    """Work list for q-blocks [b0, b1]. Each strip is one kv-block (or meta
    chunk) x a contiguous span of q columns."""
    strips = []
    lo_col = b0 * 128
    hi_col = (b1 + 1) * 128
    # meta chunks: kv block 0, dense except causal diag for q-block 0.
    col = lo_col
    while col < hi_col:
        span = min(512, hi_col - col)
        strips.append(
            dict(
                kvblk=0,
                qlo=col,
                qhi=col + span,
                meta=True,
                diag_u=0 if col == 0 else None,
                i2_u=None,
            )
        )
        col += span
    # window strips: kv block j covers q blocks {j, j+1, j+2} (j >= 1).
    for j in range(1, NB):
        i0 = max(j, b0)
        i1 = min(j + 2, b1)
        if i0 > i1:
            continue
        strips.append(
            dict(
                kvblk=j,
                qlo=i0 * 128,
                qhi=(i1 + 1) * 128,
                meta=False,
                diag_u=0 if i0 == j else None,
                i2_u=(i1 - i0) * 128 if i1 == j + 2 else None,
            )
        )
    return strips


def _pack_groups(strips):
    """Greedy hole-free packing of strips into <=1024-col group tiles such
    that every strip (and its 128-col mask sub-blocks) stays inside one
    512-col PSUM bank. Sets st['goff']. Returns [(strips, cols)]."""

    def span(s):
        return s["qhi"] - s["qlo"]

    rest = sorted(strips, key=lambda s: (-span(s), not s["meta"]))
    groups = []
    while rest:
        g, fill = [], 0
        while True:
            pick = None
            for s in rest:
                sp = span(s)
                if fill + sp <= 1024 and (fill % 512) + sp <= 512:
                    pick = s
                    break
            if pick is None:
                break
            rest.remove(pick)
            pick["goff"] = fill
            fill += span(pick)
            g.append(pick)
        assert g, "packing stuck"
        groups.append((g, fill))
    return groups


@lru_cache(maxsize=1)
def _build_program():
    import concourse.bass as bass
    import concourse.mybir as mybir
    import concourse.tile as tile
    from concourse import bacc

    bf = mybir.dt.bfloat16
    f32 = mybir.dt.float32
    EXP = mybir.ActivationFunctionType.Exp

    nc = bacc.Bacc(None)

    qt_d = nc.declare_dram_parameter("qt", [HQ_PER_CORE, D, S], bf, isOutput=False)
    kt_d = nc.declare_dram_parameter("kt", [D, S], bf, isOutput=False)
    v_d = nc.declare_dram_parameter("v", [D, NB, D], bf, isOutput=False)
    cs_d = nc.declare_dram_parameter("cs", [2, D, S], bf, isOutput=False)
    msk_d = nc.declare_dram_parameter("msk", [D, 3, 128], bf, isOutput=False)
    out_d = nc.declare_dram_parameter("out", [HQ_PER_CORE, D, S], bf, isOutput=True)

    with tile.TileContext(nc) as tc:
        with (
            tc.tile_pool(name="persist", bufs=1) as persist,
            tc.tile_pool(name="probs", bufs=4) as probs_pool,
            tc.tile_pool(name="norm", bufs=3) as norm_pool,
            tc.tile_pool(name="osb", bufs=3) as osb_pool,
            tc.tile_pool(name="grp", bufs=2, space="PSUM") as grp_psum,
            tc.tile_pool(name="acc", bufs=2, space="PSUM") as acc_psum,
        ):
            qt = persist.tile([D, HQ_PER_CORE, S], bf)
            qs = persist.tile([D, HQ_PER_CORE, S], bf)  # half-swapped q
            kt = persist.tile([D, S], bf)
            ks = persist.tile([D, S], bf)  # half-swapped k
            vt = persist.tile([D, NB, D], bf)
            cs = persist.tile([D, 2, S], bf)
            msk = persist.tile([D, 3, 128], bf)
            ones = persist.tile([D, 128], bf)
            ropek = persist.tile([D, S], bf)
            ropeq = persist.tile([D, HQ_PER_CORE, S], bf)
            ropet = persist.tile([D, 2, S], bf)

            ktr = kt_d.rearrange("d t -> d t")
            csr = cs_d.rearrange("s d t -> d s t")

            def swap_dma(dst, src, lo, hi):
                sl = slice(lo, hi)
                nc.sync.dma_start(out=dst[0:64, sl], in_=src[64:128, sl])
                nc.sync.dma_start(out=dst[64:128, sl], in_=src[0:64, sl])

            # DMA order: constants + head-0 first chunk, swaps chained off
            # the input chunks; later heads stream while head 0 computes.
            C0 = 1024
            nc.sync.dma_start(out=msk, in_=msk_d[:])
            nc.sync.dma_start(out=kt[:, :C0], in_=ktr[:, :C0])
            nc.sync.dma_start(out=cs[:, :, :C0], in_=csr[:, :, :C0])
            nc.sync.dma_start(out=qt[:, 0, :C0], in_=qt_d[0][:, :C0])
            swap_dma(ks, kt, 0, C0)
            swap_dma(qs[:, 0], qt[:, 0], 0, C0)
            nc.sync.dma_start(out=vt[:, :8], in_=v_d[:, :8])
            nc.sync.dma_start(out=kt[:, C0:], in_=ktr[:, C0:])
            nc.sync.dma_start(out=cs[:, :, C0:], in_=csr[:, :, C0:])
            nc.sync.dma_start(out=qt[:, 0, C0:], in_=qt_d[0][:, C0:])
            swap_dma(ks, kt, C0, S)
            swap_dma(qs[:, 0], qt[:, 0], C0, S)
            nc.sync.dma_start(out=vt[:, 8:], in_=v_d[:, 8:])
            for h in range(1, HQ_PER_CORE):
                nc.sync.dma_start(out=qt[:, h], in_=qt_d[h])
                swap_dma(qs[:, h], qt[:, h], 0, S)
            nc.vector.memset(ones, 1.0)

            # PE warm-up: dummy matmuls during the input-DMA wait keep the
            # HAM activity window busy so the real stream starts ramped.
            wz = acc_psum.tile([D, 512], f32, tag="zb")
            mflat = msk.rearrange("d g t -> d (g t)")
            for _ in range(16):
                nc.tensor.matmul(
                    wz[:, :384], lhsT=msk[:, 2], rhs=mflat, start=True, stop=True
                )

            def rope_k(lo, hi):
                sl = slice(lo, hi)
                nc.vector.tensor_mul(ropek[:, sl], kt[:, sl], cs[:, 0, sl])
                nc.vector.tensor_mul(ropet[:, 0, sl], ks[:, sl], cs[:, 1, sl])
                nc.vector.tensor_add(ropek[:, sl], ropek[:, sl], ropet[:, 0, sl])

            def rope_q(h, lo, hi):
                sl = slice(lo, hi)
                nc.vector.tensor_mul(ropeq[:, h, sl], qt[:, h, sl], cs[:, 0, sl])
                nc.vector.tensor_mul(ropet[:, 1, sl], qs[:, h, sl], cs[:, 1, sl])
                nc.vector.tensor_add(
                    ropeq[:, h, sl], ropeq[:, h, sl], ropet[:, 1, sl]
                )

            rope_k(0, C0)
            rope_q(0, 0, C0)
            rope_k(C0, S)
            rope_q(0, C0, S)

            def emit_qk(h, st, gp):
                """QK + additive-mask matmuls for one strip into the group
                tile at st['goff']."""
                span = st["qhi"] - st["qlo"]
                go = st["goff"]
                masks = []
                if st["diag_u"] is not None:
                    masks.append((st["diag_u"], 0))
                if st["i2_u"] is not None:
                    masks.append((st["i2_u"], 1))
                nc.tensor.matmul(
                    gp[:, go : go + span],
                    lhsT=ropek[:, st["kvblk"] * 128 : (st["kvblk"] + 1) * 128],
                    rhs=ropeq[:, h, st["qlo"] : st["qhi"]],
                    start=True,
                    stop=not masks,
                )
                for mi, (u, g) in enumerate(masks):
                    nc.tensor.matmul(
                        gp[:, go + u : go + u + 128],
                        lhsT=msk[:, 2],
                        rhs=msk[:, g],
                        start=False,
                        stop=mi == len(masks) - 1,
                    )

            def emit_back(work):
                if work[0] == "fin":
                    _, h, b0, b1, pw, zb, ot = work
                    rz = norm_pool.tile([D, 512], f32, tag="rz")
                    nc.vector.reciprocal_approx_fast(rz[:, :pw], zb[:, :pw])
                    osb = osb_pool.tile([D, 512], bf, tag="osb")
                    nc.vector.tensor_mul(osb[:, :pw], ot[:, :pw], rz[:, :pw])
                    nc.sync.dma_start(
                        out=out_d[h, :, b0 * 128 : (b1 + 1) * 128], in_=osb[:, :pw]
                    )
                    return
                _, grp, pbg, zb, ot, b0, last_id = work
                for st in grp:
                    rel = st["qlo"] - b0 * 128
                    span = st["qhi"] - st["qlo"]
                    go = st["goff"]
                    stop = id(st) == last_id
                    nc.tensor.matmul(
                        zb[:, rel : rel + span],
                        lhsT=ones,
                        rhs=pbg[:, go : go + span],
                        start=st["meta"],
                        stop=stop,
                    )
                    nc.tensor.matmul(
                        ot[:, rel : rel + span],
                        lhsT=vt[:, st["kvblk"]],
                        rhs=pbg[:, go : go + span],
                        start=st["meta"],
                        stop=stop,
                    )

            # Software-pipelined emission with a 1-group lag: PE runs QK of
            # group g+1 while ACT computes group g's exp.
            from collections import deque

            LAG = 1
            pending = deque()
            for h in range(HQ_PER_CORE):
                for pidx, (b0, b1) in enumerate(PIECES):
                    if h + 1 < HQ_PER_CORE and pidx in (1, 2):
                        half = (pidx - 1) * (S // 2)
                        rope_q(h + 1, half, half + S // 2)
                    pw = (b1 - b0 + 1) * 128
                    zb = acc_psum.tile([D, 512], f32, tag="zb")
                    ot = acc_psum.tile([D, 512], f32, tag="ot")

                    strips = _strips_for_piece(b0, b1)
                    groups = _pack_groups(strips)
                    order = [st for g, _ in groups for st in g]
                    assert order[0]["meta"]
                    last_id = id(order[-1])

                    for gi, (grp, gcols) in enumerate(groups):
                        gp = grp_psum.tile([D, 1024], f32, tag="gp")
                        pbg = probs_pool.tile([D, 1024], bf, tag="pb")
                        for st in grp:
                            emit_qk(h, st, gp)
                        nc.scalar.activation(
                            pbg[:, :gcols], gp[:, :gcols], EXP, scale=SCALE
                        )
                        pending.append(("back", grp, pbg, zb, ot, b0, last_id))
                        if gi == len(groups) - 1:
                            pending.append(("fin", h, b0, b1, pw, zb, ot))
                        while len(pending) > LAG:
                            emit_back(pending.popleft())
            while pending:
                emit_back(pending.popleft())

    nc.finalize()
    return nc


@lru_cache(maxsize=1)
def _rope_tables():
    inv_freq = 1.0 / (ROPE_BASE ** (np.arange(0, D, 2, dtype=np.float64) / D))
    pos = np.arange(S, dtype=np.float64)
    freqs = pos[:, None] * inv_freq[None, :]  # [S, 64]
    emb = np.concatenate([freqs, freqs], axis=-1)  # [S, D]
    # match the f32 reference: compute cos/sin at f32 granularity
    cosT = np.cos(emb.astype(np.float32)).T.astype(np.float32)  # [D, S]
    sinT = np.sin(emb.astype(np.float32)).T.astype(np.float32)
    sinTpm = np.concatenate([-sinT[:64], sinT[64:]], axis=0)
    return cosT, sinTpm


def _mask_tiles():
    """[128, 3, 128]: additive score masks (0 keep / -1e30 drop) for the
    causal-diag and window-tail blocks, plus a 128x128 identity (the
    stationary operand of the mask-accumulate matmuls)."""
    c = np.arange(128)[:, None]
    u = np.arange(128)[None, :]
    a_diag = np.where(u >= c, 0.0, -1e30).astype(np.float32)
    a_tail = np.where(u <= c, 0.0, -1e30).astype(np.float32)
    ident = np.eye(128, dtype=np.float32)
    return np.stack([a_diag, a_tail, ident], axis=1)  # [128, 3, 128]


def _install_ntff_shim():
    """Provide antenv.axon_hooks (NTFF profile hook) if the image lacks it,
    so run_bass_kernel_spmd(trace=True) can capture HW profiles via the
    axon PJRT .so. Silently no-ops if unavailable."""
    import sys
    import types

    try:
        from antenv.axon_hooks import get_axon_ntff_profile_hook  # noqa: F401

        return
    except ImportError:
        pass
    try:
        import contextlib
        import ctypes

        lib = ctypes.CDLL("/opt/axon/libaxon_pjrt.so")
        if not hasattr(lib, "axon_start_nrt_profile"):
            return
        lib.axon_start_nrt_profile.argtypes = [
            ctypes.POINTER(ctypes.c_int64),
            ctypes.c_size_t,
        ]
        lib.axon_start_nrt_profile.restype = ctypes.c_int64
        lib.axon_stop_nrt_profile.argtypes = [ctypes.c_char_p]
        lib.axon_stop_nrt_profile.restype = ctypes.c_int64

        @contextlib.contextmanager
        def _hook(output_dir, device_ids):
            import jax

            jax.devices()
            if device_ids:
                ids = (ctypes.c_int64 * len(device_ids))(*device_ids)
                rc = lib.axon_start_nrt_profile(ids, len(device_ids))
            else:
                rc = lib.axon_start_nrt_profile(None, 0)
            if rc != 0:
                raise RuntimeError(f"axon_start_nrt_profile rc={rc}")
            try:
                yield
            finally:
                n = lib.axon_stop_nrt_profile(str(output_dir).encode())
                print(f"ntff profile: {n} file(s) -> {output_dir}", file=sys.stderr)

        mod = types.ModuleType("antenv.axon_hooks")
        mod._hook = _hook
        mod.get_axon_ntff_profile_hook = lambda: _hook
        mod.set_axon_ntff_profile_hook = lambda h: setattr(mod, "_hook", h)
        import antenv

        antenv.axon_hooks = mod
        sys.modules["antenv.axon_hooks"] = mod
    except Exception:
        pass


def kernel(query_states, key_states, value_states):
    from concourse.bass_utils import run_bass_kernel_spmd

    _install_ntff_shim()

    nc = _build_program()

    q = np.asarray(query_states)[0]  # [S, 4096]
    k = np.asarray(key_states)[0]  # [S, 1024]
    v = np.asarray(value_states)[0]  # [S, 1024]

    cosT, sinTpm = _rope_tables()
    cs = np.stack([cosT, sinTpm], axis=0).astype(BF16)  # [2, D, S]
    msk = _mask_tiles().astype(BF16)

    in_maps = []
    for c in range(N_CORES):
        qt = np.empty((HQ_PER_CORE, D, S), dtype=BF16)
        for hh in range(HQ_PER_CORE):
            h = 4 * c + hh
            qt[hh] = np.ascontiguousarray(q[:, h * D : (h + 1) * D].T).astype(BF16)
        kt = np.ascontiguousarray(k[:, c * D : (c + 1) * D].T).astype(BF16)
        vh = v[:, c * D : (c + 1) * D]  # [S, D]
        vts = np.ascontiguousarray(
            vh.reshape(NB, 128, D).transpose(1, 0, 2)
        ).astype(BF16)  # [kv_local, j, dv]
        in_maps.append({"qt": qt, "kt": kt, "v": vts, "cs": cs, "msk": msk})

    res = run_bass_kernel_spmd(nc, in_maps, core_ids=list(range(N_CORES)))
    global LAST_RESULT
    LAST_RESULT = res

    out = np.empty((S, 32, D), dtype=np.float32)
    for c in range(N_CORES):
        o = np.asarray(res.results[c]["out"], dtype=np.float32)  # [4, D, S]
        out[:, 4 * c : 4 * c + 4, :] = o.transpose(2, 0, 1)
    return out.reshape(1, S, 32 * D)
